# revision 18
# baseline (speedup 1.0000x reference)
"""Complex-magnitude MaxPool2d (k=2, s=2) Trainium2 Bass kernel.

Input  x:  [16, 2, 64, 224, 224] f32  (plane 0 = real, plane 1 = imag)
Output:    [16, 2, 64, 112, 112] f32  (value of the window element with the
                                       largest |z|^2 = re^2 + im^2)

Sharding: pure data parallel over batch: 16 / 8 cores = 2 examples per core.
Per core the 2(batch) x 64(channel) = 128 image planes map 1:1 onto the 128
SBUF partitions; DMA moves 28 image rows at a time in a single 128-partition
dma_start (one transfer spans all 16 SBUF AXI ports and amortizes the ~2us
per-dma fixed cost); compute runs on 14-row subchunks.

Selection reproduces jnp.argmax's first-index tie-break exactly:
horizontal pass first (left/even column wins ties via is_ge), then vertical
(top row wins ties).  norm2 = fl(fl(re*re)+fl(im*im)) in f32 — ACT's Square
activation and GPSIMD's f32 add are bit-exact with the reference expression
(hardware-verified), so selections match the reference everywhere,
including exact ties.

Engine split (measured rates):
  ScalarE : squares (one ACT op per subchunk), select pre-fill copies
  GPSIMD  : norm add (in place over the squares)
  VectorE : is_ge masks + copy_predicated selects.  Masks and predicated
            dst stay contiguous (2x faster than strided), and each pred
            selects re+im together via a step-0 broadcast mask.
  DMA     : 128-partition transfers; outputs staged to long runs.
"""

import numpy as np

import concourse.bass as bass
import concourse.mybir as mybir
from concourse import bacc, bass_utils, tile

# Per-core shard geometry (hardcoded; kernel.py must be self-contained).
NCORES = 8
B = 2            # batch per core
RI = 2           # real/imag planes
C = 64           # channels
H = W = 224
HO, WO = H // 2, W // 2
P = 128          # SBUF partitions = B * C
RD = 28          # image rows per DMA chunk
R = 14           # image rows per compute subchunk
SUB = RD // R    # compute subchunks per DMA chunk (2)
NCHUNK = H // RD  # 8
N = R * W        # free elements per plane per subchunk (3136)
GROUP = 4        # subchunks staged per output store (28 output rows)
SROWS = GROUP * (R // 2)

F32 = mybir.dt.float32
I8 = mybir.dt.uint8
OP = mybir.AluOpType
ACTF = mybir.ActivationFunctionType

_NC_CACHE = []


def _build_nc() -> bass.Bass:
    nc = bacc.Bacc("TRN2", target_bir_lowering=False, debug=False)
    x = nc.dram_tensor("x", [B, RI, C, H, W], F32, kind="ExternalInput").ap()
    out = nc.dram_tensor("out", [B, RI, C, HO, WO], F32, kind="ExternalOutput").ap()

    with tile.TileContext(nc) as tc:
        with tc.tile_pool(name="pool", bufs=2) as pool:
            stage = None
            subidx = 0
            for k in range(NCHUNK):
                r0 = k * RD
                # xri free layout per partition: [ri][row 0..RD)[col]
                xri = pool.tile([P, RI * RD * W], F32, tag="xri")
                nrw = RD * W
                for ri in range(RI):
                    nc.sync.dma_start(
                        out=xri[:, ri * nrw : (ri + 1) * nrw],
                        in_=x[:, ri, :, r0 : r0 + RD, :].rearrange(
                            "b c r w -> b c (r w)"
                        ),
                    )

                for s in range(SUB):
                    # subchunk views: rows rs..rs+R of each plane
                    xri6 = xri.rearrange(
                        "p (ri r w t) -> p ri r w t", ri=RI, r=RD, w=WO, t=2
                    )[:, :, s * R : (s + 1) * R, :, :]

                    # squares of re+im rows in one ACT op; norm2 in place
                    # over the re half; im half is reused as riH below
                    sqri = pool.tile([P, RI * N], F32, tag="sqri")
                    nc.scalar.activation(
                        out=sqri.rearrange(
                            "p (ri r w t) -> p ri r w t", ri=RI, r=R, w=WO, t=2
                        ),
                        in_=xri6,
                        func=ACTF.Square,
                    )
                    nrm = sqri[:, :N]
                    nc.gpsimd.tensor_tensor(
                        out=nrm, in0=nrm, in1=sqri[:, N:], op=OP.add
                    )

                    nrm4 = nrm.rearrange("p (r w t) -> p r w t", r=R, w=WO, t=2)
                    nE, nO = nrm4[:, :, :, 0], nrm4[:, :, :, 1]

                    # horizontal mask (contiguous u8): even/left wins ties
                    cH = pool.tile([P, R * WO], I8, tag="cH")
                    cH3 = cH.rearrange("p (r w) -> p r w", r=R, w=WO)
                    nc.vector.tensor_tensor(out=cH3, in0=nE, in1=nO, op=OP.is_ge)
                    # horizontal norm max -> nrm odd slots (in place)
                    nc.vector.tensor_tensor(out=nO, in0=nE, in1=nO, op=OP.max)

                    # horizontal select of (re, im) together into the dead
                    # im-squares half: pre-fill with odd/right, overwrite
                    # where cH
                    riH = sqri[:, N:]
                    riH4 = riH.rearrange("p (ri r w) -> p ri r w", ri=RI, r=R, w=WO)
                    nc.scalar.copy(out=riH4, in_=xri6[:, :, :, :, 1])
                    cHb = cH3.unsqueeze(1).broadcast_to([P, RI, R, WO])
                    nc.vector.copy_predicated(
                        out=riH4, mask=cHb, data=xri6[:, :, :, :, 0]
                    )

                    # vertical mask from the horizontal maxes: top wins ties
                    nrm5 = nrm.rearrange(
                        "p (rp rt w t) -> p rp rt w t", rp=R // 2, rt=2, w=WO, t=2
                    )
                    cV = pool.tile([P, (R // 2) * WO], I8, tag="cV")
                    cV3 = cV.rearrange("p (rp w) -> p rp w", rp=R // 2, w=WO)
                    nc.vector.tensor_tensor(
                        out=cV3,
                        in0=nrm5[:, :, 0, :, 1],
                        in1=nrm5[:, :, 1, :, 1],
                        op=OP.is_ge,
                    )

                    # vertical select into the staged output tile
                    riH5 = riH.rearrange(
                        "p (ri rp rt w) -> p ri rp rt w",
                        ri=RI, rp=R // 2, rt=2, w=WO,
                    )
                    if subidx % GROUP == 0:
                        stage = pool.tile([P, RI * SROWS * WO], F32, tag="stage")
                    stage4 = stage.rearrange(
                        "p (ri r w) -> p ri r w", ri=RI, r=SROWS, w=WO
                    )
                    s0 = (subidx % GROUP) * (R // 2)
                    dst = stage4[:, :, s0 : s0 + R // 2, :]
                    nc.scalar.copy(out=dst, in_=riH5[:, :, :, 1, :])
                    cVb = cV3.unsqueeze(1).broadcast_to([P, RI, R // 2, WO])
                    nc.vector.copy_predicated(
                        out=dst, mask=cVb, data=riH5[:, :, :, 0, :]
                    )

                    if (subidx + 1) % GROUP == 0:
                        g0 = (subidx + 1 - GROUP) * (R // 2)
                        srw = SROWS * WO
                        for ri in range(RI):
                            nc.sync.dma_start(
                                out=out[:, ri, :, g0 : g0 + SROWS, :].rearrange(
                                    "b c r w -> b c (r w)"
                                ),
                                in_=stage[:, ri * srw : (ri + 1) * srw],
                            )
                    subidx += 1
    nc.compile()
    return nc


def get_nc() -> bass.Bass:
    if not _NC_CACHE:
        _NC_CACHE.append(_build_nc())
    return _NC_CACHE[0]


def kernel(x: np.ndarray, **run_kwargs) -> np.ndarray:
    nc = get_nc()
    xs = np.asarray(x, dtype=np.float32)
    assert xs.shape == (NCORES * B, RI, C, H, W), xs.shape
    in_maps = [{"x": xs[B * i : B * (i + 1)]} for i in range(NCORES)]
    res = bass_utils.run_bass_kernel_spmd(
        nc, in_maps, core_ids=list(range(NCORES)), **run_kwargs
    )
    out = np.concatenate([res.results[i]["out"] for i in range(NCORES)], axis=0)
    if run_kwargs:
        kernel.last_results = res
    return out


# revision 22
# speedup vs baseline: 3.9618x; 3.9618x over previous
"""Complex-magnitude MaxPool2d (k=2, s=2) Trainium2 Bass kernel.

Input  x:  [16, 2, 64, 224, 224] f32  (plane 0 = real, plane 1 = imag)
Output:    [16, 2, 64, 112, 112] f32  (value of the window element with the
                                       largest |z|^2 = re^2 + im^2)

Sharding: pure data parallel over batch: 16 / 8 cores = 2 examples per core.
Per core the 2(batch) x 64(channel) = 128 image planes map 1:1 onto the 128
SBUF partitions; DMA moves 28 image rows at a time in a single 128-partition
dma_start (one transfer spans all 16 SBUF AXI ports and amortizes the ~2us
per-dma fixed cost); compute runs on 14-row subchunks.

Selection reproduces jnp.argmax's first-index tie-break exactly:
horizontal pass first (left/even column wins ties via is_ge), then vertical
(top row wins ties).  norm2 = fl(fl(re*re)+fl(im*im)) in f32 — ACT's Square
activation and GPSIMD's f32 add are bit-exact with the reference expression
(hardware-verified), so selections match the reference everywhere,
including exact ties.

Engine split (measured rates):
  ScalarE : squares (one ACT op per subchunk), select pre-fill copies
  GPSIMD  : norm add (in place over the squares)
  VectorE : is_ge masks + copy_predicated selects.  Masks and predicated
            dst stay contiguous (2x faster than strided), and each pred
            selects re+im together via a step-0 broadcast mask.
  DMA     : 128-partition transfers; outputs staged to long runs.
"""

import numpy as np

import concourse.bass as bass
import concourse.mybir as mybir
from concourse import bacc, bass_utils, tile

# Per-core shard geometry (hardcoded; kernel.py must be self-contained).
NCORES = 8
B = 2            # batch per core
RI = 2           # real/imag planes
C = 64           # channels
H = W = 224
HO, WO = H // 2, W // 2
P = 128          # SBUF partitions = B * C
RD = 28          # image rows per DMA chunk
R = 14           # image rows per compute subchunk
SUB = RD // R    # compute subchunks per DMA chunk (2)
NCHUNK = H // RD  # 8
N = R * W        # free elements per plane per subchunk (3136)
GROUP = 4        # subchunks staged per output store (28 output rows)
SROWS = GROUP * (R // 2)

F32 = mybir.dt.float32
I8 = mybir.dt.uint8
OP = mybir.AluOpType
ACTF = mybir.ActivationFunctionType

_NC_CACHE = []


def _build_nc() -> bass.Bass:
    nc = bacc.Bacc("TRN2", target_bir_lowering=False, debug=False)
    # host pre-transposed: partition-major [b*c, ri, H, W] so every DMA is a
    # single-dim 128-partition transfer (hits all 16 SBUF AXI ports)
    x = nc.dram_tensor("x", [P, RI, H, W], F32, kind="ExternalInput").ap()
    out = nc.dram_tensor("out", [P, RI, HO, WO], F32, kind="ExternalOutput").ap()

    with tile.TileContext(nc) as tc:
        with tc.tile_pool(name="pool", bufs=2) as pool:
            stage = None
            subidx = 0
            for k in range(NCHUNK):
                r0 = k * RD
                # xri free layout per partition: [ri][row 0..RD)[col]
                xri = pool.tile([P, RI * RD * W], F32, tag="xri")
                nrw = RD * W
                nc.sync.dma_start(
                    out=xri.rearrange("p (ri f) -> p ri f", ri=RI),
                    in_=x[:, :, r0 : r0 + RD, :].rearrange("p ri r w -> p ri (r w)"),
                )

                for s in range(SUB):
                    # subchunk views: rows rs..rs+R of each plane
                    xri6 = xri.rearrange(
                        "p (ri r w t) -> p ri r w t", ri=RI, r=RD, w=WO, t=2
                    )[:, :, s * R : (s + 1) * R, :, :]

                    # squares of re+im rows in one ACT op; norm2 in place
                    # over the re half; im half is reused as riH below
                    sqri = pool.tile([P, RI * N], F32, tag="sqri")
                    nc.scalar.activation(
                        out=sqri.rearrange(
                            "p (ri r w t) -> p ri r w t", ri=RI, r=R, w=WO, t=2
                        ),
                        in_=xri6,
                        func=ACTF.Square,
                    )
                    nrm = sqri[:, :N]
                    nc.gpsimd.tensor_tensor(
                        out=nrm, in0=nrm, in1=sqri[:, N:], op=OP.add
                    )

                    nrm4 = nrm.rearrange("p (r w t) -> p r w t", r=R, w=WO, t=2)
                    nE, nO = nrm4[:, :, :, 0], nrm4[:, :, :, 1]

                    # horizontal mask (contiguous u8): even/left wins ties
                    cH = pool.tile([P, R * WO], I8, tag="cH")
                    cH3 = cH.rearrange("p (r w) -> p r w", r=R, w=WO)
                    nc.vector.tensor_tensor(out=cH3, in0=nE, in1=nO, op=OP.is_ge)
                    # horizontal norm max -> nrm odd slots (in place)
                    nc.vector.tensor_tensor(out=nO, in0=nE, in1=nO, op=OP.max)

                    # horizontal select of (re, im) together into the dead
                    # im-squares half: pre-fill with odd/right, overwrite
                    # where cH
                    riH = sqri[:, N:]
                    riH4 = riH.rearrange("p (ri r w) -> p ri r w", ri=RI, r=R, w=WO)
                    nc.scalar.copy(out=riH4, in_=xri6[:, :, :, :, 1])
                    cHb = cH3.unsqueeze(1).broadcast_to([P, RI, R, WO])
                    nc.vector.copy_predicated(
                        out=riH4, mask=cHb, data=xri6[:, :, :, :, 0]
                    )

                    # vertical mask from the horizontal maxes: top wins ties
                    nrm5 = nrm.rearrange(
                        "p (rp rt w t) -> p rp rt w t", rp=R // 2, rt=2, w=WO, t=2
                    )
                    cV = pool.tile([P, (R // 2) * WO], I8, tag="cV")
                    cV3 = cV.rearrange("p (rp w) -> p rp w", rp=R // 2, w=WO)
                    nc.vector.tensor_tensor(
                        out=cV3,
                        in0=nrm5[:, :, 0, :, 1],
                        in1=nrm5[:, :, 1, :, 1],
                        op=OP.is_ge,
                    )

                    # vertical select into the staged output tile
                    riH5 = riH.rearrange(
                        "p (ri rp rt w) -> p ri rp rt w",
                        ri=RI, rp=R // 2, rt=2, w=WO,
                    )
                    if subidx % GROUP == 0:
                        stage = pool.tile([P, RI * SROWS * WO], F32, tag="stage")
                    stage4 = stage.rearrange(
                        "p (ri r w) -> p ri r w", ri=RI, r=SROWS, w=WO
                    )
                    s0 = (subidx % GROUP) * (R // 2)
                    dst = stage4[:, :, s0 : s0 + R // 2, :]
                    nc.scalar.copy(out=dst, in_=riH5[:, :, :, 1, :])
                    cVb = cV3.unsqueeze(1).broadcast_to([P, RI, R // 2, WO])
                    nc.vector.copy_predicated(
                        out=dst, mask=cVb, data=riH5[:, :, :, 0, :]
                    )

                    if (subidx + 1) % GROUP == 0:
                        g0 = (subidx + 1 - GROUP) * (R // 2)
                        nc.sync.dma_start(
                            out=out[:, :, g0 : g0 + SROWS, :].rearrange(
                                "p ri r w -> p ri (r w)"
                            ),
                            in_=stage.rearrange("p (ri f) -> p ri f", ri=RI),
                        )
                    subidx += 1
    nc.compile()
    return nc


def get_nc() -> bass.Bass:
    if not _NC_CACHE:
        _NC_CACHE.append(_build_nc())
    return _NC_CACHE[0]


def kernel(x: np.ndarray, **run_kwargs) -> np.ndarray:
    nc = get_nc()
    xs = np.asarray(x, dtype=np.float32)
    assert xs.shape == (NCORES * B, RI, C, H, W), xs.shape
    # [16,2,64,H,W] -> per core [b,c,ri,H,W] flattened to [128,ri,H,W]
    xt = np.ascontiguousarray(xs.transpose(0, 2, 1, 3, 4))
    in_maps = [
        {"x": xt[B * i : B * (i + 1)].reshape(P, RI, H, W)} for i in range(NCORES)
    ]
    res = bass_utils.run_bass_kernel_spmd(
        nc, in_maps, core_ids=list(range(NCORES)), **run_kwargs
    )
    # per-core [128,ri,HO,WO] -> [b,c,ri,HO,WO] -> [b,ri,c,HO,WO]
    out = np.concatenate(
        [
            res.results[i]["out"].reshape(B, C, RI, HO, WO).transpose(0, 2, 1, 3, 4)
            for i in range(NCORES)
        ],
        axis=0,
    )
    if run_kwargs:
        kernel.last_results = res
    return np.ascontiguousarray(out)
